# revision 66
# baseline (speedup 1.0000x reference)
"""Trainium2 Bass kernel: gumbel-softmax-argmax embedding lookup (end-to-end).

Reference math (nn_End2End_49495203119139):
    hot  = argmax_V(softmax((logits + gumbel)/tau))  == argmax_V(logits+gumbel)
    row  = grid_sample-nearest index map of hot      == ROWMAP[hot]  (LUT)
    inputs_embeds = W[row] * mask          (col map == arange(E), verified)
    psg branch: roll/flip/rotate of psg ids, flag = cumsum(trunc!=0) > 0,
    out = inputs_embeds + where(flag, W[trunc], 0)

Key structural fact (verified exactly in numpy, dev_check.py): the attention
mask is a contiguous run of len_b ones then zeros, which makes the two
branches DISJOINT per position:
    l <  len_b: out[b,l] = W[ROWMAP[argmax_V(logits+gumbel)[b,l]]]
    l >= len_b: out[b,l] = W[psg_roll[b, l-len_b]],
                psg_roll[0] = 1 (BOS), psg_roll[j] = psg[b, j-1]
so only the sum(len_b) ACTIVE positions need the memory-bound vocab stream.
For the canonical input that is 1419/2048 positions (69%).

Sharding: the active positions are resharded EVENLY across the 8 cores
(Ta = ceil(A/8) vocab-streams per core instead of 256 batch-sharded ones),
which cuts the per-core HBM traffic from 66 MB to ~2*Ta*125.5KB + gathers.
The inactive positions are a pure indirect W-row gather (Tp per core).

Per-core layout trick: each active token's 32128-float logits row is folded
host-side to [128, 251] (partition-major), and a core's Ta tokens are
concatenated along the free axis -> DRAM [128*Ta, 251] viewed as
[128, Ta*251].  Every streaming DMA is then a full-width 128-partition
transfer regardless of Ta (a partial-partition tile would run at the same
wall time as a full one - the slowest SBUF port binds).

Device plan per core:
  - psg phase (independent of streaming, issued first): load Tp host-computed
    W-row ids, one indirect gather, store.
  - stream phase: chunks of 8 tokens = [128, 2008] f32. HWDGE(sync ring)
    loads logits chunk; SWDGE CCE-add DMA accumulates the gumbel chunk
    in the DMA datapath (one <=2048-element descriptor per partition);
    ONE 3D-AP DVE reduce_max per chunk -> per-token strip maxes into the
    stats tile mx[128, 128] (partition p = vocab fold strip, col = token).
  - per 64-token granule tail, split into phases emitted after successive
    chunks so the in-order SWDGE/DVE queues never wait on tail deps:
    ph0: exact DVE 32x32 stream-transposes -> token-major mxTg; max/
    max_index -> winning strip p* (first occurrence == argmax tie-break);
    ph1a: two indirect gathers re-fetch the winning [1,251] logits/gumbel
    strips (rows p**Ta+t and P*Ta+p**Ta+t of lgg); ph1b: DVE add +
    max/max_index -> within-strip c*; hot = p**251 + c*; ph2: indirect
    gather of wrm = W[rowmap] (precomposed host-side, so no LUT hop);
    ph3: store rows on the scalar HWDGE ring (never stalls chunk loads).
    Only the last granule's chain is exposed after streaming ends (~11us);
    everything else hides under the DMA stream.

Cost model (TimelineSim, matches the harness metric): per-core stream holds
the DMA engines 45.7MB/360GB/s = 131.4us; tails+psg add ~6us of DMA holds
and the end chain ~11us of latency -> 149us total vs 212us baseline.

Host does only sharding/unsharding: selecting + reordering rows by the mask
(data-dependent sharding), the [128,251] fold, and scattering the returned
rows into [16,128,768]. All arithmetic on tensor VALUES happens on device.

Tie-breaking matches jnp argmax (first occurrence) exactly: vocab index =
p*251+c with strips in vocab order; DVE max_index returns the first index;
cross-partition winner is the first partition attaining the max; CCE f32
add == DVE f32 add bitwise, so streamed maxes and re-gathered strips agree.
"""

import numpy as np

B = 16
L = 128
V = 32128
E = 768
N_CORES = 8
P = 128                   # partitions; V = P * C
C = V // P                # 251 columns per folded strip
TPC = 8                   # tokens per streamed chunk -> 2008 cols <= 2048
NEG = -3.0e38


def _build(nc_mod, ta, tp, variant=None):
    """Build the per-core Bass module for ta active streams + tp psg rows."""
    import concourse.tile as tile
    from concourse import bass, mybir
    from concourse.bass import IndirectOffsetOnAxis

    var = variant or {}

    nc = nc_mod
    f32 = mybir.dt.float32
    i32 = mybir.dt.int32
    u32 = mybir.dt.uint32
    Op = mybir.AluOpType
    AX = mybir.AxisListType

    n_blk = (ta + P - 1) // P

    # logits fold rows [0, P*ta), gumbel fold rows [P*ta, 2*P*ta): one tensor
    # so the strip re-gathers share a single gather source
    lgg_h = nc.dram_tensor("lgg", [2 * P * ta, C], f32, kind="ExternalInput")
    # wrm = W[rowmap] precomposed host-side (rowmap is a pure LUT of V)
    wrm_h = nc.dram_tensor("wrm", [V, E], f32, kind="ExternalInput")
    out_act_h = nc.dram_tensor("out_act", [ta, E], f32, kind="ExternalOutput")
    if tp:
        wte_h = nc.dram_tensor("wte", [V, E], f32, kind="ExternalInput")
        psgidx_h = nc.dram_tensor("psgidx", [tp, 1], i32, kind="ExternalInput")
        out_psg_h = nc.dram_tensor("out_psg", [tp, E], f32, kind="ExternalOutput")

    # [128, ta*251] streaming views (row-major contiguous reshapes)
    lg2 = lgg_h[0:P * ta, :].rearrange("(p t) c -> p (t c)", p=P)
    gm2 = lgg_h[P * ta:2 * P * ta, :].rearrange("(p t) c -> p (t c)", p=P)

    with tile.TileContext(nc) as tc:
        with (
            tc.tile_pool(name="lpool", bufs=var.get("LBUFS", 8)) as lpool,
            tc.tile_pool(name="stats", bufs=4 * n_blk) as stats,
            tc.tile_pool(name="small", bufs=3) as small,
            tc.tile_pool(name="emb", bufs=2) as emb,
            tc.tile_pool(name="consts", bufs=1) as consts,
        ):
            # ---- constants ----
            iota_p_i = consts.tile([P, 1], i32)
            nc.gpsimd.iota(iota_p_i[:], pattern=[[1, 1]], base=0, channel_multiplier=1)
            iota_pf = consts.tile([P, 1], f32)
            nc.vector.tensor_copy(out=iota_pf[:], in_=iota_p_i[:])
            idx_mode = var.get("IDX_MODE", False)
            # track in-stream argmax indices for the LAST block only: its
            # granule resolves after the final chunk, and skipping the strip
            # re-gather there cuts ~5us off the exposed end chain, while the
            # extra per-chunk max_index work hides under streaming
            idx_last = var.get("IDX_LAST", False)
            if idx_mode or idx_last:
                # per-partition row 0..127 along the free axis (strip ids)
                iota_c_i = consts.tile([P, P], i32)
                nc.gpsimd.iota(iota_c_i[:], pattern=[[1, P]], base=0,
                               channel_multiplier=0)
                iota_cols = consts.tile([P, P], f32)
                nc.vector.tensor_copy(out=iota_cols[:], in_=iota_c_i[:])

            # ---- psg phase: pure indirect W gather, overlaps streaming ----
            for p0 in range(0, tp, P):
                pn = min(P, tp - p0)
                pidx = small.tile([pn, 1], i32, tag="pidx")
                nc.scalar.dma_start(out=pidx[:], in_=psgidx_h[p0:p0 + pn, :])
                pemb = emb.tile([pn, E], f32, tag="pemb")
                nc.gpsimd.indirect_dma_start(
                    out=pemb[:], out_offset=None, in_=wte_h[:],
                    in_offset=IndirectOffsetOnAxis(ap=pidx[:, 0:1], axis=0))
                nc.scalar.dma_start(out=out_psg_h[p0:p0 + pn, :], in_=pemb[:])

            S = 32                            # transpose block size
            GS = var.get("GS", 64)            # granule: tokens per tail set

            def granule_phases(b, g, mx, ix=None, gsz=None):
                """Phases resolving tokens [b*128+GS*g, +GS): each later
                phase's work depends only on phases issued >= one chunk
                earlier, so the in-order SWDGE/DVE queues never stall the
                streaming on tail dependencies."""
                gsz = gsz or GS
                t0b = b * P
                nt = min(P, ta - t0b)
                lo = g * gsz                     # local token range [lo, hi)
                hi = min(lo + gsz, nt)
                gs = hi - lo                     # real tokens (<= gsz)
                gsp = min(gsz, P - lo)           # padded partition extent
                st = {}

                def ph0():
                    # transpose mx[:, lo:lo+gsp] into a base-0 token-major
                    # tile (exact 32x32 copies) and resolve p* per token.
                    # All compute tiles sit at partition base 0: the BIR
                    # verifier requires equal base partitions for two-SB-input
                    # instructions (NCC_IBIR297).
                    mxTg = stats.tile([gsp, P], f32, tag="mxTg")
                    for j in range(gsp // S):
                        for i in range(P // S):
                            nc.vector.transpose(
                                out=mxTg[S * j:S * (j + 1), S * i:S * i + S],
                                in_=mx[S * i:S * i + S,
                                       lo + S * j:lo + S * (j + 1)])
                    if ix is not None:
                        # transpose the strided col-0-of-8 view of ix
                        ix3 = ix[:, :].rearrange("p (t e) -> p t e", e=8)
                        ixTg = stats.tile([gsp, P], u32, tag="ixTg")
                        for j in range(gsp // S):
                            for i in range(P // S):
                                nc.vector.transpose(
                                    out=ixTg[S * j:S * (j + 1), S * i:S * i + S],
                                    in_=ix3[S * i:S * i + S,
                                            lo + S * j:lo + S * (j + 1), 0])
                        st["ixTg"] = ixTg
                    gmax8 = small.tile([gsp, 8], f32, tag="gmax8")
                    nc.vector.max(out=gmax8[:], in_=mxTg[:])
                    p8 = small.tile([gsp, 8], u32, tag="p8")
                    nc.vector.max_index(out=p8[:], in_max=gmax8[:], in_values=mxTg[:])
                    p1f = small.tile([gsp, 1], f32, tag="p1f")
                    nc.vector.tensor_copy(out=p1f[:], in_=p8[:, 0:1])
                    # token ids of this granule, at partition base 0
                    tofs = small.tile([gsp, 1], f32, tag="tofs")
                    nc.vector.tensor_scalar(tofs[:], iota_pf[0:gsp],
                                            float(b * P + lo), None, op0=Op.add)
                    # fold rows of the winning strips in lgg: col0 = logits
                    # half (p*ta + t), col1 = gumbel half (+ P*ta)
                    rows2 = small.tile([gsp, 2], f32, tag="rows2")
                    nc.vector.scalar_tensor_tensor(
                        out=rows2[:, 0:1], in0=p1f[:], scalar=float(ta),
                        in1=tofs[:], op0=Op.mult, op1=Op.add)
                    nc.vector.tensor_scalar(rows2[:, 1:2], rows2[:, 0:1],
                                            float(P * ta), None, op0=Op.add)
                    rows2i = small.tile([gsp, 2], i32, tag="rows2i")
                    nc.vector.tensor_copy(out=rows2i[:], in_=rows2[:])
                    st["p1f"], st["rows2i"] = p1f, rows2i

                def ph0_idx():
                    # c* from the in-stream index stats: select column p* of
                    # ixTg (one masked multiply + reduce), no strip re-fetch
                    ixTf = small.tile([gsp, P], f32, tag="ixTf")
                    nc.vector.tensor_copy(out=ixTf[:], in_=st["ixTg"][:])
                    selx = small.tile([gsp, P], f32, tag="selx")
                    nc.vector.scalar_tensor_tensor(
                        out=selx[:], in0=iota_cols[0:gsp, :],
                        scalar=st["p1f"][:, 0:1], in1=ixTf[:],
                        op0=Op.is_equal, op1=Op.mult)
                    c1f = small.tile([gsp, 1], f32, tag="c1f")
                    nc.vector.reduce_max(out=c1f[:], in_=selx[:], axis=AX.X)
                    hotf = small.tile([gsp, 1], f32, tag="hotf")
                    nc.vector.scalar_tensor_tensor(
                        out=hotf[:], in0=st["p1f"], scalar=float(C), in1=c1f[:],
                        op0=Op.mult, op1=Op.add)
                    hot_i = small.tile([gsp, 1], i32, tag="hot_i")
                    nc.vector.tensor_copy(out=hot_i[:], in_=hotf[:])
                    st["hot_i"] = hot_i

                def ph1a():
                    # two single-index gathers fetch the winning logits and
                    # gumbel strips. (A fused 2-index gather - ap=[:,0:2],
                    # out [gsp, 2C] - simulates per-index in CoreSim, but HW
                    # ignores the second index and fetches CONSECUTIVE rows
                    # idx0, idx0+1 - probed on device; keep them separate.)
                    stl = emb.tile([gsp, 2 * C], f32, tag="stl")
                    nc.gpsimd.indirect_dma_start(
                        out=stl[:, 0:C], out_offset=None, in_=lgg_h[:],
                        in_offset=IndirectOffsetOnAxis(
                            ap=st["rows2i"][:, 0:1], axis=0))
                    nc.gpsimd.indirect_dma_start(
                        out=stl[:, C:2 * C], out_offset=None, in_=lgg_h[:],
                        in_offset=IndirectOffsetOnAxis(
                            ap=st["rows2i"][:, 1:2], axis=0))
                    st["stl"] = stl

                def ph1b():
                    # recompute l+g on the fetched strips, find c*
                    stl = st["stl"]
                    ssum = emb.tile([gsp, C], f32, tag="ssum")
                    nc.vector.tensor_tensor(out=ssum[:], in0=stl[:, 0:C],
                                            in1=stl[:, C:2 * C], op=Op.add)
                    s8 = small.tile([gsp, 8], f32, tag="s8")
                    nc.vector.max(out=s8[:], in_=ssum[:])
                    c8 = small.tile([gsp, 8], u32, tag="c8")
                    nc.vector.max_index(out=c8[:], in_max=s8[:], in_values=ssum[:])
                    c1f = small.tile([gsp, 1], f32, tag="c1f")
                    nc.vector.tensor_copy(out=c1f[:], in_=c8[:, 0:1])
                    hotf = small.tile([gsp, 1], f32, tag="hotf")
                    nc.vector.scalar_tensor_tensor(
                        out=hotf[:], in0=st["p1f"], scalar=float(C), in1=c1f[:],
                        op0=Op.mult, op1=Op.add)
                    hot_i = small.tile([gsp, 1], i32, tag="hot_i")
                    nc.vector.tensor_copy(out=hot_i[:], in_=hotf[:])
                    st["hot_i"] = hot_i

                def ph2():
                    # W[rowmap[.]] is precomposed host-side into wrm
                    wrows = emb.tile([gsp, E], f32, tag="wrows")
                    nc.gpsimd.indirect_dma_start(
                        out=wrows[:], out_offset=None, in_=wrm_h[:],
                        in_offset=IndirectOffsetOnAxis(ap=st["hot_i"][:, 0:1],
                                                       axis=0))
                    st["wrows"] = wrows

                def ph3():
                    # scalar ring: never stalls the chunk loads on sync
                    nc.scalar.dma_start(out=out_act_h[t0b + lo:t0b + hi, :],
                                        in_=st["wrows"][0:gs, :])

                if ix is not None:
                    phases = [ph0, ph0_idx, ph2, ph3]
                else:
                    phases = [ph0, ph1a, ph1b, ph2, ph3]
                return phases[:var.get("TAIL_LEVEL", 9)]

            # chunk schedule across all blocks, then interleave tail phases
            tpc_v = var.get("TPC", TPC)
            chunks = []          # (b, t0, tn)
            for b in range(n_blk):
                t0b = b * P
                nt = min(P, ta - t0b)
                for t0 in range(t0b, t0b + nt, tpc_v):
                    tn = min(tpc_v, t0b + nt - t0)
                    chunks.append((b, t0, tn))

            blk_tiles = {}
            pending = []         # (due_chunk_idx, phase_fn)
            gran_seen = set()
            for ci, (b, t0, tn) in enumerate(chunks):
                t0b = b * P
                nt = min(P, ta - t0b)
                track_ix = idx_mode or (idx_last and b == n_blk - 1)
                if b not in blk_tiles:
                    mx = stats.tile([P, P], f32, tag=f"mx{b}")
                    ix = None
                    if track_ix:
                        # 8-wide per token: max_index outputs land in place
                        ix = stats.tile([P, P * 8], u32, tag=f"ix{b}")
                    if nt < P:
                        nc.vector.memset(mx[:], NEG)
                        if track_ix:
                            nc.vector.memset(ix[:], 0)
                    blk_tiles[b] = (mx, ix)
                mx, ix = blk_tiles[b]
                cols = tn * C
                tpc = var.get("TPC", TPC)
                lt = lpool.tile([P, tpc * C], f32, tag="lt")
                ldeng = nc.scalar if (var.get("DUAL_HWDGE") and ci % 2) else nc.sync
                ldeng.dma_start(out=lt[:, 0:cols],
                                in_=lg2[:, t0 * C:(t0 + tn) * C])
                if not var.get("SKIP_ACCUM"):
                    # CCE-add descriptors must stay <= 2048 elements on HW:
                    # split the accumulate at 8-token granularity
                    for a0 in range(0, tn, TPC):
                        an = min(TPC, tn - a0)
                        nc.gpsimd.dma_start(
                            out=lt[:, a0 * C:(a0 + an) * C],
                            in_=gm2[:, (t0 + a0) * C:(t0 + a0 + an) * C],
                            accum_op=Op.add)
                if not var.get("SKIP_REDUCE"):
                    col = t0 - t0b
                    if var.get("UNBATCH_REDUCE"):
                        for j in range(tn):
                            nc.vector.reduce_max(out=mx[:, col + j:col + j + 1],
                                                 in_=lt[:, j * C:(j + 1) * C],
                                                 axis=AX.X)
                    else:
                        # all tn per-token strip maxes in ONE 3D-AP reduce
                        nc.vector.reduce_max(
                            out=mx[:, col:col + tn],
                            in_=lt[:, 0:cols].rearrange("p (t c) -> p t c", c=C),
                            axis=AX.X)
                    if track_ix:
                        # within-strip argmax per token, tracked in-stream;
                        # max_index writes its 8-wide result straight into
                        # the widened stats tile (no copy op)
                        for j in range(tn):
                            nc.vector.max_index(
                                out=ix[:, (col + j) * 8:(col + j + 1) * 8],
                                in_max=mx[:, col + j:col + j + 1].to_broadcast([P, 8]),
                                in_values=lt[:, j * C:(j + 1) * C])
                elif t0 + tn >= t0b + nt:
                    nc.vector.reduce_max(out=mx[:, 0:1],
                                         in_=lt[:, 0:C], axis=AX.X)
                if var.get("SKIP_TAILS"):
                    continue
                # queue tail phases for granules completed by this chunk;
                # the LAST block uses finer granules so most of its tail
                # resolves before the final chunk (shorter end flush)
                gsz = var.get("LAST_GS", GS) if b == n_blk - 1 else GS
                streamed = t0 - t0b + tn
                for g in range((nt + gsz - 1) // gsz):
                    if (b, g) in gran_seen:
                        continue
                    if streamed >= min((g + 1) * gsz, nt):
                        gran_seen.add((b, g))
                        dues = var.get("PH_DUES",
                                       (0, 1, 2, 3) if idx_mode else (0, 1, 2, 3, 4))
                        phs = granule_phases(b, g, mx, ix, gsz)
                        for k, ph in enumerate(phs):
                            pending.append((ci + dues[k], ph))
                # emit everything due after this chunk, in phase order
                due = [x for x in pending if x[0] <= ci]
                pending = [x for x in pending if x[0] > ci]
                for _, ph in due:
                    ph()
            if var.get("SKIP_FLUSH"):
                pending = []
            for _, ph in sorted(pending, key=lambda x: x[0]):
                ph()

    return nc


_BUILD_CACHE = {}


def _get_module(ta, tp, variant=None):
    key = (ta, tp, tuple(sorted((variant or {}).items())))
    if key not in _BUILD_CACHE:
        import concourse.bacc as bacc

        nc = bacc.Bacc("TRN2", target_bir_lowering=False, debug=False)
        _build(nc, ta, tp, variant)
        nc.compile()
        _BUILD_CACHE[key] = nc
    return _BUILD_CACHE[key]


# The reference's f32 grid_sample-nearest index maps, precomputed with jnp
# (the backend the reference runs on) for the hardcoded V=32128 / E=768:
# the column map is exactly identity; the row map is identity except at
# these 17 indices (f32 rounding of the normalized-coordinate roundtrip).
_ROWMAP_DIFF_IDX = [1, 2, 6, 11, 16, 32079, 32089, 32093, 32099, 32103,
                    32107, 32109, 32113, 32117, 32119, 32121, 32123]
_ROWMAP_DIFF_VAL = [0, 1, 5, 10, 15, 32080, 32090, 32094, 32100, 32104,
                    32108, 32110, 32114, 32118, 32120, 32122, 32124]


def _nearest_maps():
    rowmap = np.arange(V, dtype=np.int32)
    rowmap[_ROWMAP_DIFF_IDX] = _ROWMAP_DIFF_VAL
    return rowmap, np.arange(E, dtype=np.int32)


# test/dev hooks: set TRACE=True before calling kernel() to capture an NTFF
# profile; the BassKernelResults of the last run is stored in LAST_RESULT.
TRACE = False
LAST_RESULT = None
LAST_MODULE = None
DEFAULT_VARIANT = None   # dev hook: build-variant dict used by kernel()


def _fold(rows):
    """[n, V] f32 -> [128*n, 251] partition-major fold."""
    n = rows.shape[0]
    return np.ascontiguousarray(
        rows.reshape(n, P, C).transpose(1, 0, 2).reshape(P * n, C))


def kernel(logits, rwrt_attention_mask, psg_input_ids, word_embeddings, gumbel_noise):
    from concourse.bass_utils import run_bass_kernel_spmd

    logits = np.ascontiguousarray(np.asarray(logits, dtype=np.float32)).reshape(B * L, V)
    gumbel = np.ascontiguousarray(np.asarray(gumbel_noise, dtype=np.float32)).reshape(B * L, V)
    mask = np.asarray(rwrt_attention_mask, dtype=np.int32)
    psg = np.asarray(psg_input_ids, dtype=np.int32)
    wte = np.ascontiguousarray(np.asarray(word_embeddings, dtype=np.float32))

    # wrm = W[rowmap] precomposed (rowmap is identity except 17 rows)
    wrm = wte.copy()
    wrm[_ROWMAP_DIFF_IDX] = wte[_ROWMAP_DIFF_VAL]

    lens = mask.sum(axis=1)
    contiguous = bool(np.all(mask == (np.arange(L)[None, :] < lens[:, None])))

    if contiguous:
        # fast path: the two branches are positionally disjoint (see header)
        act_pos = []           # flat b*L+l, in output order
        psg_pos = []
        psg_rows = []
        for b in range(B):
            ln = int(lens[b])
            act_pos.extend(b * L + l for l in range(ln))
            for l in range(ln, L):
                psg_pos.append(b * L + l)
                psg_rows.append(1 if l == ln else int(psg[b, l - ln - 1]))
    else:
        # general fallback (never taken for the reference's inputs): stream
        # every position's argmax on device, gather both branches' W rows on
        # device, combine per the reference's mask/flag weights at unshard
        # time. Index arithmetic below mirrors the reference exactly.
        act_pos = list(range(B * L))
        psg_roll = np.roll(psg, 1, axis=1)
        psg_roll[:, 0] = 1
        extr = (1 - mask[:, ::-1]) * psg_roll
        pos = (np.arange(L)[None, :] - lens[:, None]) % L
        trunc = np.take_along_axis(extr, pos, axis=1)
        flag = (np.cumsum(trunc != 0, axis=1) > 0).astype(np.float32)
        psg_pos = list(range(B * L))
        psg_rows = trunc.reshape(-1).tolist()
    A, Pn = len(act_pos), len(psg_pos)
    ta = max(1, (A + N_CORES - 1) // N_CORES)
    tp = (Pn + N_CORES - 1) // N_CORES

    pad_src = act_pos[-1] if act_pos else 0
    act_idx = np.asarray(act_pos + [pad_src] * (ta * N_CORES - A), np.int64)
    psg_idx = np.asarray(psg_rows + [0] * (tp * N_CORES - Pn), np.int32)

    nc = _get_module(ta, tp, DEFAULT_VARIANT)
    global LAST_MODULE
    LAST_MODULE = nc

    in_maps = []
    for m in range(N_CORES):
        sl = act_idx[m * ta:(m + 1) * ta]
        im = {
            "lgg": np.concatenate([_fold(logits[sl]), _fold(gumbel[sl])], axis=0),
            "wrm": wrm,
        }
        if tp:
            im["wte"] = wte
            im["psgidx"] = np.ascontiguousarray(
                psg_idx[m * tp:(m + 1) * tp].reshape(tp, 1))
        in_maps.append(im)

    global LAST_RESULT
    try:
        LAST_RESULT = run_bass_kernel_spmd(nc, in_maps, list(range(N_CORES)), trace=TRACE)
    except Exception:
        # the axon-relayed device occasionally reports a transient
        # NRT_EXEC_UNIT_UNRECOVERABLE on the first execution after long
        # sessions; a straight re-run recovers it
        import time as _time

        _time.sleep(2.0)
        LAST_RESULT = run_bass_kernel_spmd(nc, in_maps, list(range(N_CORES)), trace=TRACE)
    res = LAST_RESULT.results

    acts = np.concatenate([res[m]["out_act"] for m in range(N_CORES)], axis=0)
    if contiguous:
        out = np.empty((B * L, E), np.float32)
        out[np.asarray(act_pos, np.int64)] = acts[:A]
        if Pn:
            psgs = np.concatenate(
                [res[m]["out_psg"] for m in range(N_CORES)], axis=0)
            out[np.asarray(psg_pos, np.int64)] = psgs[:Pn]
        return out.reshape(B, L, E)
    psgs = np.concatenate([res[m]["out_psg"] for m in range(N_CORES)], axis=0)
    out = (acts[:A] * mask.reshape(-1, 1)
           + psgs[:Pn] * flag.reshape(-1, 1)).astype(np.float32)
    return out.reshape(B, L, E)
